# revision 9
# baseline (speedup 1.0000x reference)
"""Supervised-contrastive point-cloud loss on Trainium2 (8 NeuronCores).

Inputs (full): features [8, 128, 4096] f32, labels_all [8, 4096] int32.
Sharding: data-parallel over the batch dim - core b computes the
per-cloud loss pieces for cloud b; the host does the O(N) tail.

Symmetric-half algorithm (per core). Points are class-sorted on the host
into 16 contiguous slots of common (cross-cloud) size S_c, padded with
zero feature columns; N_pad = sum(S_c) rounded up to 128. dp = exp(10*G)
is symmetric, so only upper-triangle 128-row blocks are computed:
  for R = NB-1 .. 0:
    G[j in R, i in [128R, N_pad)] = v_R^T v      (bf16 matmuls, PSUM f32)
    diagonal killed in-psum by an extra (320*I)^T(-320*I) accumulate
    dp row = exp(10*G)                           (ACT, bf16 out, only half!)
    row side:  pos/tot via DVE ranged reductions over class slots
    mirror side: for C > R: matmul with dp tile as the STATIONARY operand
      (lhsT=dp_RC [j,i], rhs=Y_R [j,16]) accumulating CS^T[i in C, 16]
      into per-block [128,16] PSUM tiles - symmetry gives the lower half
      without computing or exp-ing it.
Pad columns give dp=1 contributions the host subtracts exactly (counts
are known); pad rows carry Y=0 so the mirror side ignores them.
Host: pos = row + mirror parts, tot = sum over classes; loss =
mean(ln tot - ln pos) over real points, mean over clouds.
"""

import contextlib
import sys

for _p in ("/opt/trn_rl_repo",):
    if _p not in sys.path:
        sys.path.append(_p)

import numpy as np
import ml_dtypes

import concourse.bass as bass  # noqa: F401
import concourse.bacc as bacc
import concourse.tile as tile
from concourse import mybir
from concourse.bass_utils import run_bass_kernel_spmd

F32 = mybir.dt.float32
BF16 = mybir.dt.bfloat16
AF = mybir.ActivationFunctionType
ALU = mybir.AluOpType
AX = mybir.AxisListType

B, C, N = 8, 128, 4096
NCLS = 16
TEMP_INV = 10.0
DIAG_A = 320.0  # bf16-exact; 320^2 = 102400 subtracted on the diagonal


def _layout(labels_all):
    """Common class-slot layout across the 8 clouds."""
    labels = np.asarray(labels_all, dtype=np.int64)
    counts = np.stack([np.bincount(labels[b], minlength=NCLS) for b in range(B)])
    S = counts.max(axis=0)  # [16] common slot sizes
    S = S + (S & 1)  # even slot sizes: keeps every DVE reduce slice 4B-aligned
    O = np.concatenate([[0], np.cumsum(S)])  # slot offsets, O[16] = sum S
    n_used = int(O[NCLS])
    n_pad = -(-n_used // 128) * 128
    return counts, S.astype(int), O.astype(int), n_pad


class _Prog:
    def __init__(self, nc, n_pad, spans):
        self.nc = nc
        self.n_pad = n_pad
        self.spans = spans  # (R, c, lo, hi) DVE reduce spans (global cols)


def build_program(n_pad, class_offsets, class_sizes):
    NP = n_pad
    NB = NP // 128
    KCH = [(k * 512, min(k * 512 + 512, NP)) for k in range(-(-NP // 512))]

    nc = bacc.Bacc("TRN2", target_bir_lowering=False, debug=False, num_devices=B)

    f_d = nc.dram_tensor("f", [C, NP], F32, kind="ExternalInput").ap()
    y16_d = nc.dram_tensor("y16", [C, NB * NCLS], BF16, kind="ExternalInput").ap()
    eyep_d = nc.dram_tensor("eyep", [128, 128], BF16, kind="ExternalInput").ap()
    eyen_d = nc.dram_tensor("eyen", [128, 128], BF16, kind="ExternalInput").ap()
    onescol_d = nc.dram_tensor("onescol", [128, 1], F32, kind="ExternalInput").ap()
    onesrow_d = nc.dram_tensor("onesrow", [1, 128], F32, kind="ExternalInput").ap()
    rowcs_d = nc.dram_tensor("rowcs", [128, NB * NCLS], F32, kind="ExternalOutput").ap()
    colcs_d = nc.dram_tensor("colcs", [128, 1024], F32, kind="ExternalOutput").ap()

    # DVE reduce spans: per row-block R, class ranges clipped to [128R, NP)
    spans = []
    for R in range(NB):
        base = 128 * R
        for c in range(NCLS):
            lo = max(int(class_offsets[c]), base)
            hi = int(class_offsets[c]) + int(class_sizes[c])
            if hi > lo:
                spans.append((R, c, lo, hi))

    with tile.TileContext(nc) as tc, contextlib.ExitStack() as _stack:
        with (
            tc.tile_pool(name="const", bufs=1) as constp,
            tc.tile_pool(name="big", bufs=1) as bigp,
            tc.tile_pool(name="dprow", bufs=3) as dpp,
        ):
            y16_sb = constp.tile([C, NB * NCLS], BF16)
            nc.sync.dma_start(y16_sb[:], y16_d[:])
            eyep_sb = constp.tile([128, 128], BF16)
            nc.sync.dma_start(eyep_sb[:], eyep_d[:])
            eyen_sb = constp.tile([128, 128], BF16)
            nc.sync.dma_start(eyen_sb[:], eyen_d[:])
            onescol_sb = constp.tile([128, 1], F32)
            nc.sync.dma_start(onescol_sb[:], onescol_d[:])
            onesrow_sb = constp.tile([1, 128], F32)
            nc.sync.dma_start(onesrow_sb[:], onesrow_d[:])
            tiny_sb = constp.tile([1, 1], F32)
            nc.gpsimd.memset(tiny_sb[:], 1e-30)

            f_sb = bigp.tile([C, NP], F32)
            fsq = bigp.tile([C, NP], F32)
            v_sb = bigp.tile([C, NP], BF16)
            lnrow = bigp.tile([1, NP], F32)
            rnrow = bigp.tile([1, NP], F32)
            rowcs_sb = bigp.tile([128, NB * NCLS], F32)
            colcs_sb = bigp.tile([128, 1024], F32)

            # ---- prologue: normalize columns, pipelined in REVERSE chunk
            # order so high row-blocks (which need only the column suffix)
            # unlock immediately.
            with tc.tile_pool(name="pmisc", bufs=2, space="PSUM") as pmiscp:
                for k0, k1 in reversed(KCH):
                    w = k1 - k0
                    nc.sync.dma_start(f_sb[:, k0:k1], f_d[:, k0:k1])
                    nc.vector.tensor_tensor(
                        fsq[:, k0:k1], f_sb[:, k0:k1], f_sb[:, k0:k1], op=ALU.mult
                    )
                    s2_ps = pmiscp.tile([1, 512], F32, tag="pm")
                    nc.tensor.matmul(
                        s2_ps[:, :w], onescol_sb[:], fsq[:, k0:k1],
                        start=True, stop=True,
                    )
                    nc.scalar.activation(
                        lnrow[0:1, k0:k1], s2_ps[:, :w], AF.Ln,
                        bias=tiny_sb[0:1, 0:1],
                    )
                # second phase: Exp for all chunks, so each ACT table set
                # loads once instead of thrashing ~2.7us per switch per chunk
                for k0, k1 in reversed(KCH):
                    w = k1 - k0
                    nc.scalar.activation(
                        rnrow[0:1, k0:k1], lnrow[0:1, k0:k1], AF.Exp, scale=-0.5
                    )
                    bc_ps = pmiscp.tile([128, 512], F32, tag="pm2")
                    nc.tensor.matmul(
                        bc_ps[:, :w], onesrow_sb[:], rnrow[0:1, k0:k1],
                        start=True, stop=True,
                    )
                    nc.vector.tensor_tensor(
                        v_sb[:, k0:k1], f_sb[:, k0:k1], bc_ps[:, :w], op=ALU.mult
                    )

            # ---- main loop over row-blocks, descending ----
            pgp = _stack.enter_context(tc.tile_pool(name="pg", bufs=2, space="PSUM"))
            pcsp = _stack.enter_context(tc.tile_pool(name="pcs", bufs=1, space="PSUM"))
            pcs = pcsp.tile([128, 1024], F32)

            dprows = {}  # R -> (tile, base_col)
            cs_pending = []  # rows whose mirror-side matmuls are not yet emitted
            # start=True clears has_written for the WHOLE psum bank, which
            # would wipe sibling slice accumulators - so emit exactly one
            # start per bank (the chronologically first matmul into it).
            # start=False overwrites where bits are clear (each slice's first
            # touch) and accumulates where set.
            started_banks = set()

            def emit_cs(R):
                dp_t, base = dprows.pop(R)
                lhs_all = dp_t
                for Cb in range(R + 1, NB):
                    off = 128 * Cb - base
                    bank = (Cb * NCLS) // 512
                    nc.tensor.matmul(
                        pcs[:, Cb * NCLS : (Cb + 1) * NCLS],
                        lhs_all[:, off : off + 128],
                        y16_sb[:, R * NCLS : (R + 1) * NCLS],
                        start=(bank not in started_banks),
                        stop=(R == 0),
                        skip_group_check=True,
                    )
                    started_banks.add(bank)

            for R in range(NB - 1, -1, -1):
                base = 128 * R
                W = NP - base
                dp_t = dpp.tile([128, NP], BF16, tag="dp")
                dprows[R] = (dp_t, base)

                # G pieces of up to 1536 psum columns
                pieces = []
                p0 = base
                while p0 < NP:
                    p1 = min(p0 + 1536, NP)
                    pieces.append((p0, p1))
                    p0 = p1

                for pi, (p0, p1) in enumerate(pieces):
                    g = pgp.tile([128, 1536], F32, tag="g")
                    # matmul chunks of <=512 within the piece; the diagonal
                    # 128 columns (first 128 of the first piece) get the
                    # -320^2*I accumulate.
                    q0 = p0
                    while q0 < p1:
                        q1 = min(q0 + 512, p1)
                        if pi == 0 and q0 == base:
                            # diag group: [base, base+128) two-matmul group
                            nc.tensor.matmul(
                                g[:, 0:128],
                                eyep_sb[:],
                                eyen_sb[:],
                                start=True, stop=False,
                            )
                            nc.tensor.matmul(
                                g[:, 0:128],
                                v_sb[:, base : base + 128],
                                v_sb[:, base : base + 128],
                                start=False, stop=True,
                            )
                            if q1 > base + 128:
                                nc.tensor.matmul(
                                    g[:, 128 : q1 - p0],
                                    v_sb[:, base : base + 128],
                                    v_sb[:, base + 128 : q1],
                                    start=True, stop=True,
                                )
                        else:
                            nc.tensor.matmul(
                                g[:, q0 - p0 : q1 - p0],
                                v_sb[:, base : base + 128],
                                v_sb[:, q0:q1],
                                start=True, stop=True,
                            )
                        q0 = q1
                    nc.scalar.activation(
                        dp_t[:, p0 - base : p1 - base],
                        g[:, 0 : p1 - p0],
                        AF.Exp,
                        scale=TEMP_INV,
                    )

                # row-side ranged class sums on DVE
                for (Rs, c, lo, hi) in ((s for s in spans if s[0] == R)):
                    nc.vector.tensor_reduce(
                        rowcs_sb[:, R * NCLS + c : R * NCLS + c + 1],
                        dp_t[:, lo - base : hi - base],
                        axis=AX.X,
                        op=ALU.add,
                    )

                # mirror side of the PREVIOUS row (its exp finished while this
                # row's G was streaming) - keeps the PE from stalling on ACT.
                cs_pending.append(R)
                if len(cs_pending) > 1:
                    emit_cs(cs_pending.pop(0))
            for R in cs_pending:
                emit_cs(R)

            # retire PSUM cs accumulators + DMA outputs
            nc.vector.tensor_copy(colcs_sb[:, 0:512], pcs[:, 0:512])
            nc.vector.tensor_copy(colcs_sb[:, 512:1024], pcs[:, 512:1024])
            nc.sync.dma_start(colcs_d[:], colcs_sb[:])
            nc.sync.dma_start(rowcs_d[:], rowcs_sb[:])

    nc.compile()
    return _Prog(nc, NP, spans)


_PROG = None
_PROG_KEY = None


def _get_program(labels_all):
    global _PROG, _PROG_KEY
    counts, S, O, n_pad = _layout(labels_all)
    key = (n_pad, tuple(S))
    if _PROG is None or _PROG_KEY != key:
        _PROG = build_program(n_pad, O, S)
        _PROG_KEY = key
    return _PROG, counts, S, O, n_pad


def make_in_maps(features, labels_all, counts, S, O, n_pad):
    feats = np.asarray(features, dtype=np.float32)
    labels = np.asarray(labels_all, dtype=np.int64)
    NP = n_pad
    NB = NP // 128

    slot_of = np.zeros((B, N), dtype=np.int64)
    fpad = np.zeros((B, C, NP), dtype=np.float32)
    y16 = np.zeros((B, NP, NCLS), dtype=ml_dtypes.bfloat16)
    for b in range(B):
        for c in range(NCLS):
            idx = np.nonzero(labels[b] == c)[0]
            s = np.arange(len(idx)) + O[c]
            slot_of[b, idx] = s
            fpad[b][:, s] = feats[b][:, idx]
            y16[b][s, c] = 1.0
    # [NP, 16] -> [128, NB*16] block-major so Y_R slices are [128, 16]
    y16p = np.ascontiguousarray(
        y16.reshape(B, NB, 128, NCLS).transpose(0, 2, 1, 3).reshape(B, 128, NB * NCLS)
    )
    eyep = (np.eye(128) * DIAG_A).astype(ml_dtypes.bfloat16)
    eyen = (np.eye(128) * -DIAG_A).astype(ml_dtypes.bfloat16)
    onescol = np.ones((128, 1), np.float32)
    onesrow = np.ones((1, 128), np.float32)
    in_maps = [
        {
            "f": np.ascontiguousarray(fpad[b]),
            "y16": y16p[b],
            "eyep": eyep,
            "eyen": eyen,
            "onescol": onescol,
            "onesrow": onesrow,
        }
        for b in range(B)
    ]
    return in_maps, slot_of


def finish_on_host(rowcs_all, colcs_all, labels_all, counts, S, O, n_pad, slot_of):
    labels = np.asarray(labels_all, dtype=np.int64)
    NP = n_pad
    NB = NP // 128
    losses = []
    for b in range(B):
        rowcs = np.asarray(rowcs_all[b], dtype=np.float64)  # [128, NB*16]
        colcs = np.asarray(colcs_all[b], dtype=np.float64)  # [128, 1024]
        # per-block [128, 16] views
        rowm = rowcs.reshape(128, NB, NCLS).transpose(1, 0, 2).copy()  # [NB,128,16]
        colm = colcs[:, : NB * NCLS].reshape(128, NB, NCLS).transpose(1, 0, 2).copy()
        colm[0] = 0.0  # block 0 has no mirror contributions (never written)
        # pad-column corrections for the row side: class c pads occupy
        # [O_c + n_bc, O_c + S_c); block R's reduce covers [max(O_c,128R), ..)
        corr = np.zeros((NB, NCLS))
        valid = np.zeros((NB, NCLS), dtype=bool)
        for R in range(NB):
            base = 128 * R
            for c in range(NCLS):
                lo = max(O[c] + counts[b, c], base)
                hi = O[c] + S[c]
                corr[R, c] = max(0, hi - lo)
                valid[R, c] = hi > base  # device emitted a reduce for (R, c)
        rowm = np.where(valid[:, None, :], rowm - corr[:, None, :], 0.0)
        per = rowm + colm  # [NB, 128, 16]
        s = slot_of[b]
        blk, prt = s >> 7, s & 127
        pos = per[blk, prt, labels[b]]
        tot = per[blk, prt, :].sum(axis=1)
        dev = np.log(tot) - np.log(pos)
        losses.append(dev.mean())
    return np.asarray(np.float32(np.mean(losses)))


def run(features, labels_all, **spmd_kwargs):
    prog, counts, S, O, n_pad = _get_program(labels_all)
    in_maps, slot_of = make_in_maps(features, labels_all, counts, S, O, n_pad)
    res = run_bass_kernel_spmd(prog.nc, in_maps, list(range(B)), **spmd_kwargs)
    out = finish_on_host(
        [res.results[b]["rowcs"] for b in range(B)],
        [res.results[b]["colcs"] for b in range(B)],
        labels_all, counts, S, O, n_pad, slot_of,
    )
    return out, res


def kernel(features, labels_all):
    out, _ = run(features, labels_all)
    return out
